# revision 24
# baseline (speedup 1.0000x reference)
"""Trainium2 Bass kernel for nn_BroadcastEdgeUpdate.

reference computes:
    res_edge_index = flat_atom_res_index[edge_index]           # [2, E]
    flatish_z      = z.reshape(R, n_res, c_z)                  # R = n_batch*n_res
    update         = einsum('rsc,ac->rsa', LN(flatish_z), W)   # [R, n_res, 16]
    out            = update[res_edge_index[0], res_edge_index[1] % n_res]

Sharding: core i owns table rows r0 in [64*i, 64*i+64) (z first-dim shard).
Edges are bucketed on the host by r0-block so each core gathers only from
its own locally-computed 2 MB table slice; the host undoes the permutation.

Device pipeline per core:
  phase A: z slice [32768, 128] --DMA--> bn_stats (DVE) -> rstd (ACT sqrt +
           DVE recip) -> fused (x-mu)*r (DVE tensor_scalar) -> PE transpose ->
           PE matmul with Wg = gamma*W^T -> PE transpose back -> +beta@W^T ->
           row-major [32768, 16] table in DRAM
  phase B: 1056 indirect-DMA instructions, each gathering 128 table rows
           (one 64 B row per partition, walrus consumes one offset per
           partition), batched into contiguous DMA-outs.
           (dma_gather/DMAGatherAnt would do 8k rows/inst but silently moves
           no data on this axon/fake_nrt runtime; ap_gather works but runs
           ~110 ns/idx on the Q7s — both rejected.)
"""

import numpy as np

import concourse.bass as bass
import concourse.bacc as bacc
import concourse.mybir as mybir
import concourse.tile as tile
from concourse import bass_utils
from concourse.bass import IndirectOffsetOnAxis

N_CORES = 8
N_RES = 512
C_Z = 128
C_AP = 16
ROWS_PER_CORE = (N_RES // N_CORES) * N_RES  # 32768 table rows
K_TOT = 1056                                # gather instructions per core
N_PAD = K_TOT * 128                         # 135168 padded edges per core
GB = 16                                     # gather insts batched per output DMA
SG_ROWS = 4096                              # rows per super-group (32 tiles)
N_SG = ROWS_PER_CORE // SG_ROWS             # 8
LN_EPS = 1e-5
DEBUG_TABLE = False

_prog_cache = {}


def _build_program():
    f32 = mybir.dt.float32
    i32 = mybir.dt.int32
    nc = bacc.Bacc("TRN2", target_bir_lowering=False, debug=False,
                   num_devices=N_CORES)

    zs = nc.dram_tensor("zs", [ROWS_PER_CORE, C_Z], f32, kind="ExternalInput").ap()
    wg = nc.dram_tensor("wg", [C_Z, C_AP], f32, kind="ExternalInput").ap()
    bw64 = nc.dram_tensor("bw64", [128, 4 * C_AP], f32, kind="ExternalInput").ap()
    ident = nc.dram_tensor("ident", [128, 128], f32, kind="ExternalInput").ap()
    eidx = nc.dram_tensor("eidx", [128, K_TOT], i32, kind="ExternalInput").ap()
    # out[p, k*16 + c] = gathered edge  j = k*128 + p
    out = nc.dram_tensor("out", [128, K_TOT * C_AP], f32,
                         kind="ExternalOutput").ap()
    table_dbg = (nc.dram_tensor("table_dbg", [ROWS_PER_CORE, C_AP], f32,
                                kind="ExternalOutput").ap()
                 if DEBUG_TABLE else None)

    with tile.TileContext(nc) as tc:
        with (
            tc.tile_pool(name="const", bufs=1) as cpool,
            tc.tile_pool(name="xin", bufs=2) as xpool,
            tc.tile_pool(name="xn", bufs=2) as xnpool,
            tc.tile_pool(name="stat", bufs=2) as spool,
            tc.tile_pool(name="xnt", bufs=3) as tpool,
            tc.tile_pool(name="u", bufs=3) as upool,
            tc.tile_pool(name="ostage", bufs=2) as opool,
            tc.tile_pool(name="psumT", bufs=2, space="PSUM") as ptpool,
            tc.tile_pool(name="psumA", bufs=2, space="PSUM") as papool,
            tc.tile_pool(name="psum2", bufs=2, space="PSUM") as p2pool,
            tc.tile_pool(name="gidx", bufs=1) as gipool,
            tc.tile_pool(name="gout", bufs=4) as gopool,
            tc.tile_pool(name="tbl", bufs=1, space="DRAM") as dpool,
        ):
            wg_t = cpool.tile([C_Z, C_AP], f32)
            nc.sync.dma_start(out=wg_t[:], in_=wg[:, :])
            bw_t = cpool.tile([128, 4 * C_AP], f32)
            nc.sync.dma_start(out=bw_t[:], in_=bw64[:, :])
            id_t = cpool.tile([128, 128], f32)
            nc.sync.dma_start(out=id_t[:], in_=ident[:, :])

            table = dpool.tile([ROWS_PER_CORE, C_AP], f32)

            # ---------------- phase A: build the update table ----------------
            for sg in range(N_SG):
                x = xpool.tile([128, 32, C_Z], f32, tag="x")
                rows = zs[sg * SG_ROWS:(sg + 1) * SG_ROWS, :]
                nc.sync.dma_start(out=x[:], in_=rows.rearrange("(t p) c -> p t c", p=128))

                stats = spool.tile([128, 32, 6], f32, tag="stats")
                for t in range(32):
                    nc.vector.bn_stats(out=stats[:, t, :], in_=x[:, t, :])

                # combine even/odd stats: n=128, ce=co=64
                # var = (M2e + M2o + 32*(me-mo)^2)/128 ; mean = (me+mo)/2
                t1 = spool.tile([128, 32, 1], f32, tag="t1")
                t2 = spool.tile([128, 32, 1], f32, tag="t2")
                t3 = spool.tile([128, 32, 1], f32, tag="t3")
                sd = spool.tile([128, 32, 1], f32, tag="sd")
                rr = spool.tile([128, 32, 1], f32, tag="rr")
                ms = spool.tile([128, 32, 1], f32, tag="ms")
                nc.vector.tensor_tensor(out=t1[:], in0=stats[:, :, 1:2],
                                        in1=stats[:, :, 4:5],
                                        op=mybir.AluOpType.subtract)
                nc.vector.tensor_tensor(out=t2[:], in0=stats[:, :, 2:3],
                                        in1=stats[:, :, 5:6],
                                        op=mybir.AluOpType.add)
                nc.vector.tensor_tensor(out=t3[:], in0=t1[:], in1=t1[:],
                                        op=mybir.AluOpType.mult)
                # t3 <- 32*t3 + 128*eps, then += t2  == 128*(var + eps)
                nc.vector.tensor_scalar(out=t3[:], in0=t3[:], scalar1=32.0,
                                        scalar2=float(C_Z * LN_EPS),
                                        op0=mybir.AluOpType.mult,
                                        op1=mybir.AluOpType.add)
                nc.vector.tensor_tensor(out=t3[:], in0=t3[:], in1=t2[:],
                                        op=mybir.AluOpType.add)
                nc.scalar.activation(out=sd[:], in_=t3[:],
                                     func=mybir.ActivationFunctionType.Sqrt,
                                     bias=0.0, scale=1.0 / C_Z)
                nc.vector.reciprocal(out=rr[:], in_=sd[:])
                nc.vector.tensor_tensor(out=ms[:], in0=stats[:, :, 1:2],
                                        in1=stats[:, :, 4:5],
                                        op=mybir.AluOpType.add)
                nc.vector.tensor_scalar(out=ms[:], in0=ms[:], scalar1=0.5,
                                        scalar2=None, op0=mybir.AluOpType.mult)

                xn = xnpool.tile([128, 32, C_Z], f32, tag="xn")
                for t in range(32):
                    nc.vector.tensor_scalar(out=xn[:, t, :], in0=x[:, t, :],
                                            scalar1=ms[:, t, :],
                                            scalar2=rr[:, t, :],
                                            op0=mybir.AluOpType.subtract,
                                            op1=mybir.AluOpType.mult)

                ostage = opool.tile([128, 32, C_AP], f32, tag="ostage")
                for gg in range(8):
                    psum_t = ptpool.tile([128, 512], f32, tag="pt")
                    for t4 in range(4):
                        nc.tensor.transpose(out=psum_t[:, 128 * t4:128 * (t4 + 1)],
                                            in_=xn[:, 4 * gg + t4, :],
                                            identity=id_t[:])
                    xnt = tpool.tile([128, 512], f32, tag="xnt")
                    if gg % 2 == 0:
                        nc.vector.tensor_copy(out=xnt[:], in_=psum_t[:])
                    else:
                        nc.scalar.copy(out=xnt[:], in_=psum_t[:])
                    psum_a = papool.tile([C_AP, 512], f32, tag="pa")
                    nc.tensor.matmul(out=psum_a[:], lhsT=wg_t[:], rhs=xnt[:],
                                     start=True, stop=True)
                    u = upool.tile([C_AP, 512], f32, tag="u")
                    nc.scalar.copy(out=u[:], in_=psum_a[:])
                    psum_2 = p2pool.tile([128, 4 * C_AP], f32, tag="p2")
                    for t4 in range(4):
                        nc.tensor.transpose(out=psum_2[:, C_AP * t4:C_AP * (t4 + 1)],
                                            in_=u[:, 128 * t4:128 * (t4 + 1)],
                                            identity=id_t[:C_AP, :C_AP])
                    nc.vector.tensor_tensor(out=ostage[:, 4 * gg:4 * gg + 4, :],
                                            in0=psum_2[:].rearrange("p (t c) -> p t c", t=4),
                                            in1=bw_t[:].rearrange("p (t c) -> p t c", t=4),
                                            op=mybir.AluOpType.add)
                rows_out = table[sg * SG_ROWS:(sg + 1) * SG_ROWS, :]
                nc.sync.dma_start(
                    out=rows_out.rearrange("(t p) c -> p t c", p=128),
                    in_=ostage[:])
                if table_dbg is not None:
                    dbg_rows = table_dbg[sg * SG_ROWS:(sg + 1) * SG_ROWS, :]
                    nc.sync.dma_start(
                        out=dbg_rows.rearrange("(t p) c -> p t c", p=128),
                        in_=ostage[:])

            # ---------------- phase B: gather the edges ----------------
            # indirect DMA: 128 rows (one per partition) per instruction
            idx_all = gipool.tile([128, K_TOT], i32)
            nc.sync.dma_start(out=idx_all[:], in_=eidx[:, :])
            for b in range(K_TOT // GB):
                g = gopool.tile([128, GB, C_AP], f32, tag="gout")
                for t in range(GB):
                    k = b * GB + t
                    nc.gpsimd.indirect_dma_start(
                        out=g[:, t, :],
                        out_offset=None,
                        in_=table[:, :],
                        in_offset=IndirectOffsetOnAxis(
                            ap=idx_all[:, k:k + 1], axis=0),
                    )
                nc.sync.dma_start(
                    out=out[:, GB * C_AP * b:GB * C_AP * (b + 1)],
                    in_=g[:].rearrange("p t c -> p (t c)"),
                )

    nc.compile()
    return nc


def _get_program():
    if "nc" not in _prog_cache:
        _prog_cache["nc"] = _build_program()
    return _prog_cache["nc"]


def kernel(z, ln_gamma, ln_beta, W, flat_atom_res_index, edge_index):
    z = np.asarray(z)
    ln_gamma = np.asarray(ln_gamma, dtype=np.float32)
    ln_beta = np.asarray(ln_beta, dtype=np.float32)
    W = np.asarray(W, dtype=np.float32)
    fari = np.asarray(flat_atom_res_index)
    ei = np.asarray(edge_index)

    n_batch, n_res, _, c_z = z.shape
    assert (n_batch, n_res, c_z) == (1, N_RES, C_Z)
    n_edges = ei.shape[1]

    zf = np.ascontiguousarray(z, dtype=np.float32).reshape(n_batch * n_res * n_res, c_z)

    # ------- host: constants -------
    wg = np.ascontiguousarray((ln_gamma[:, None] * W.T).astype(np.float32))  # [128,16]
    bw = (ln_beta @ W.T).astype(np.float32)                                  # [16]
    bw64 = np.ascontiguousarray(np.tile(bw, (128, 4)).astype(np.float32))    # [128,64]
    ident = np.eye(128, dtype=np.float32)

    # ------- host: bucket edges by r0-block -------
    r0 = fari[ei[0]].astype(np.int64)
    r1 = (fari[ei[1]].astype(np.int64)) % n_res
    core_of = (r0 >> 6).astype(np.int64)          # 64 rows per core
    order = np.argsort(core_of, kind="stable")
    counts = np.bincount(core_of, minlength=N_CORES)
    starts = np.zeros(N_CORES + 1, dtype=np.int64)
    np.cumsum(counts, out=starts[1:])

    r_local = ((r0 & 63) * n_res + r1).astype(np.int32)   # [0, 32768)

    in_maps = []
    edge_ids = []
    overflow = []
    for c in range(N_CORES):
        sel = order[starts[c]:starts[c + 1]]
        if len(sel) > N_PAD:
            overflow.append(sel[N_PAD:])
            sel = sel[:N_PAD]
        edge_ids.append(sel)
        ib = np.zeros(N_PAD, dtype=np.int32)
        ib[:len(sel)] = r_local[sel]
        # edge j = k*128 + p  ->  eidx[p, k]
        in_maps.append({
            "zs": np.ascontiguousarray(zf[c * ROWS_PER_CORE:(c + 1) * ROWS_PER_CORE]),
            "wg": wg,
            "bw64": bw64,
            "ident": ident,
            "eidx": np.ascontiguousarray(ib.reshape(K_TOT, 128).T),
        })

    nc = _get_program()
    res = bass_utils.run_bass_kernel_spmd(nc, in_maps, core_ids=list(range(N_CORES)))
    global _LAST_RES
    _LAST_RES = res

    out_full = np.empty((n_edges, C_AP), dtype=np.float32)
    for c in range(N_CORES):
        sel = edge_ids[c]
        # device out: [128, K_TOT*16]; edge j = k*128 + p at out[p, k*16:+16]
        dv = res.results[c]["out"].reshape(128, K_TOT, C_AP)
        dv = dv.transpose(1, 0, 2).reshape(N_PAD, C_AP)
        out_full[sel] = dv[:len(sel)]

    # host fallback for bucket overflow (normally empty)
    for sel in overflow:
        rows = zf[r0[sel] * n_res + r1[sel]].astype(np.float64)
        mu = rows.mean(axis=1, keepdims=True)
        var = rows.var(axis=1)
        xn = (rows - mu) / np.sqrt(var + LN_EPS)[:, None]
        out_full[sel] = (xn @ wg.astype(np.float64) + bw).astype(np.float32)

    return out_full


# revision 27
# speedup vs baseline: 1.0271x; 1.0271x over previous
"""Trainium2 Bass kernel for nn_BroadcastEdgeUpdate.

reference computes:
    res_edge_index = flat_atom_res_index[edge_index]           # [2, E]
    flatish_z      = z.reshape(R, n_res, c_z)                  # R = n_batch*n_res
    update         = einsum('rsc,ac->rsa', LN(flatish_z), W)   # [R, n_res, 16]
    out            = update[res_edge_index[0], res_edge_index[1] % n_res]

Sharding: core i owns table rows r0 in [64*i, 64*i+64) (z first-dim shard).
Edges are bucketed on the host by r0-block so each core gathers only from
its own locally-computed 2 MB table slice; the host undoes the permutation.

Device pipeline per core:
  phase A: z slice [32768, 128] --DMA--> bn_stats (DVE) -> rstd (ACT sqrt +
           DVE recip) -> fused (x-mu)*r (DVE tensor_scalar) -> PE transpose ->
           PE matmul with Wg = gamma*W^T -> PE transpose back -> +beta@W^T ->
           row-major [32768, 16] table in DRAM
  phase B: 1056 indirect-DMA instructions, each gathering 128 table rows
           (one 64 B row per partition, walrus consumes one offset per
           partition), batched into contiguous DMA-outs.
           (dma_gather/DMAGatherAnt would do 8k rows/inst but silently moves
           no data on this axon/fake_nrt runtime; ap_gather works but runs
           ~110 ns/idx on the Q7s — both rejected.)
"""

import numpy as np

import concourse.bass as bass
import concourse.bacc as bacc
import concourse.mybir as mybir
import concourse.tile as tile
from concourse import bass_utils
from concourse.bass import IndirectOffsetOnAxis

N_CORES = 8
N_RES = 512
C_Z = 128
C_AP = 16
ROWS_PER_CORE = (N_RES // N_CORES) * N_RES  # 32768 table rows
K_TOT = 1024                                # gather instructions per core
N_PAD = K_TOT * 128                         # 135168 padded edges per core
GB = 16                                     # gather insts batched per output DMA
SG_ROWS = 4096                              # rows per super-group (32 tiles)
N_SG = ROWS_PER_CORE // SG_ROWS             # 8
LN_EPS = 1e-5
DEBUG_TABLE = False

_prog_cache = {}


def _build_program():
    f32 = mybir.dt.float32
    i32 = mybir.dt.int32
    nc = bacc.Bacc("TRN2", target_bir_lowering=False, debug=False,
                   num_devices=N_CORES)

    zs = nc.dram_tensor("zs", [ROWS_PER_CORE, C_Z], f32, kind="ExternalInput").ap()
    wg = nc.dram_tensor("wg", [C_Z, C_AP], f32, kind="ExternalInput").ap()
    bw64 = nc.dram_tensor("bw64", [128, 4 * C_AP], f32, kind="ExternalInput").ap()
    ident = nc.dram_tensor("ident", [128, 128], f32, kind="ExternalInput").ap()
    eidx = nc.dram_tensor("eidx", [128, K_TOT], i32, kind="ExternalInput").ap()
    # out[p, k*16 + c] = gathered edge  j = k*128 + p
    out = nc.dram_tensor("out", [128, K_TOT * C_AP], f32,
                         kind="ExternalOutput").ap()
    table_dbg = (nc.dram_tensor("table_dbg", [ROWS_PER_CORE, C_AP], f32,
                                kind="ExternalOutput").ap()
                 if DEBUG_TABLE else None)

    with tile.TileContext(nc) as tc:
        with (
            tc.tile_pool(name="const", bufs=1) as cpool,
            tc.tile_pool(name="xin", bufs=2) as xpool,
            tc.tile_pool(name="xn", bufs=2) as xnpool,
            tc.tile_pool(name="stat", bufs=2) as spool,
            tc.tile_pool(name="xnt", bufs=3) as tpool,
            tc.tile_pool(name="u", bufs=3) as upool,
            tc.tile_pool(name="ostage", bufs=2) as opool,
            tc.tile_pool(name="psumT", bufs=2, space="PSUM") as ptpool,
            tc.tile_pool(name="psumA", bufs=2, space="PSUM") as papool,
            tc.tile_pool(name="psum2", bufs=2, space="PSUM") as p2pool,
            tc.tile_pool(name="gidx", bufs=1) as gipool,
            tc.tile_pool(name="gout", bufs=4) as gopool,
            tc.tile_pool(name="tbl", bufs=1, space="DRAM") as dpool,
        ):
            wg_t = cpool.tile([C_Z, C_AP], f32)
            nc.sync.dma_start(out=wg_t[:], in_=wg[:, :])
            bw_t = cpool.tile([128, 4 * C_AP], f32)
            nc.sync.dma_start(out=bw_t[:], in_=bw64[:, :])
            id_t = cpool.tile([128, 128], f32)
            nc.sync.dma_start(out=id_t[:], in_=ident[:, :])

            table = dpool.tile([ROWS_PER_CORE, C_AP], f32)

            # ---------------- phase A: build the update table ----------------
            for sg in range(N_SG):
                x = xpool.tile([128, 32, C_Z], f32, tag="x")
                rows = zs[sg * SG_ROWS:(sg + 1) * SG_ROWS, :]
                nc.sync.dma_start(out=x[:], in_=rows.rearrange("(t p) c -> p t c", p=128))

                stats = spool.tile([128, 32, 6], f32, tag="stats")
                for t in range(32):
                    nc.vector.bn_stats(out=stats[:, t, :], in_=x[:, t, :])

                # combine even/odd stats: n=128, ce=co=64
                # var = (M2e + M2o + 32*(me-mo)^2)/128 ; mean = (me+mo)/2
                t1 = spool.tile([128, 32, 1], f32, tag="t1")
                t2 = spool.tile([128, 32, 1], f32, tag="t2")
                t3 = spool.tile([128, 32, 1], f32, tag="t3")
                sd = spool.tile([128, 32, 1], f32, tag="sd")
                rr = spool.tile([128, 32, 1], f32, tag="rr")
                ms = spool.tile([128, 32, 1], f32, tag="ms")
                nc.vector.tensor_tensor(out=t1[:], in0=stats[:, :, 1:2],
                                        in1=stats[:, :, 4:5],
                                        op=mybir.AluOpType.subtract)
                nc.vector.tensor_tensor(out=t2[:], in0=stats[:, :, 2:3],
                                        in1=stats[:, :, 5:6],
                                        op=mybir.AluOpType.add)
                nc.vector.tensor_tensor(out=t3[:], in0=t1[:], in1=t1[:],
                                        op=mybir.AluOpType.mult)
                # t3 <- 32*t3 + 128*eps, then += t2  == 128*(var + eps)
                nc.vector.tensor_scalar(out=t3[:], in0=t3[:], scalar1=32.0,
                                        scalar2=float(C_Z * LN_EPS),
                                        op0=mybir.AluOpType.mult,
                                        op1=mybir.AluOpType.add)
                nc.vector.tensor_tensor(out=t3[:], in0=t3[:], in1=t2[:],
                                        op=mybir.AluOpType.add)
                nc.scalar.activation(out=sd[:], in_=t3[:],
                                     func=mybir.ActivationFunctionType.Sqrt,
                                     bias=0.0, scale=1.0 / C_Z)
                nc.vector.reciprocal(out=rr[:], in_=sd[:])
                nc.vector.tensor_tensor(out=ms[:], in0=stats[:, :, 1:2],
                                        in1=stats[:, :, 4:5],
                                        op=mybir.AluOpType.add)
                nc.vector.tensor_scalar(out=ms[:], in0=ms[:], scalar1=0.5,
                                        scalar2=None, op0=mybir.AluOpType.mult)

                xn = xnpool.tile([128, 32, C_Z], f32, tag="xn")
                for t in range(32):
                    nc.vector.tensor_scalar(out=xn[:, t, :], in0=x[:, t, :],
                                            scalar1=ms[:, t, :],
                                            scalar2=rr[:, t, :],
                                            op0=mybir.AluOpType.subtract,
                                            op1=mybir.AluOpType.mult)

                ostage = opool.tile([128, 32, C_AP], f32, tag="ostage")
                for gg in range(8):
                    psum_t = ptpool.tile([128, 512], f32, tag="pt")
                    for t4 in range(4):
                        nc.tensor.transpose(out=psum_t[:, 128 * t4:128 * (t4 + 1)],
                                            in_=xn[:, 4 * gg + t4, :],
                                            identity=id_t[:])
                    xnt = tpool.tile([128, 512], f32, tag="xnt")
                    if gg % 2 == 0:
                        nc.vector.tensor_copy(out=xnt[:], in_=psum_t[:])
                    else:
                        nc.scalar.copy(out=xnt[:], in_=psum_t[:])
                    psum_a = papool.tile([C_AP, 512], f32, tag="pa")
                    nc.tensor.matmul(out=psum_a[:], lhsT=wg_t[:], rhs=xnt[:],
                                     start=True, stop=True)
                    u = upool.tile([C_AP, 512], f32, tag="u")
                    nc.scalar.copy(out=u[:], in_=psum_a[:])
                    psum_2 = p2pool.tile([128, 4 * C_AP], f32, tag="p2")
                    for t4 in range(4):
                        nc.tensor.transpose(out=psum_2[:, C_AP * t4:C_AP * (t4 + 1)],
                                            in_=u[:, 128 * t4:128 * (t4 + 1)],
                                            identity=id_t[:C_AP, :C_AP])
                    nc.vector.tensor_tensor(out=ostage[:, 4 * gg:4 * gg + 4, :],
                                            in0=psum_2[:].rearrange("p (t c) -> p t c", t=4),
                                            in1=bw_t[:].rearrange("p (t c) -> p t c", t=4),
                                            op=mybir.AluOpType.add)
                rows_out = table[sg * SG_ROWS:(sg + 1) * SG_ROWS, :]
                nc.sync.dma_start(
                    out=rows_out.rearrange("(t p) c -> p t c", p=128),
                    in_=ostage[:])
                if table_dbg is not None:
                    dbg_rows = table_dbg[sg * SG_ROWS:(sg + 1) * SG_ROWS, :]
                    nc.sync.dma_start(
                        out=dbg_rows.rearrange("(t p) c -> p t c", p=128),
                        in_=ostage[:])

            # ---------------- phase B: gather the edges ----------------
            # indirect DMA: 128 rows (one per partition) per instruction
            idx_all = gipool.tile([128, K_TOT], i32)
            nc.sync.dma_start(out=idx_all[:], in_=eidx[:, :])
            for b in range(K_TOT // GB):
                g = gopool.tile([128, GB, C_AP], f32, tag="gout")
                for t in range(GB):
                    k = b * GB + t
                    nc.gpsimd.indirect_dma_start(
                        out=g[:, t, :],
                        out_offset=None,
                        in_=table[:, :],
                        in_offset=IndirectOffsetOnAxis(
                            ap=idx_all[:, k:k + 1], axis=0),
                    )
                nc.sync.dma_start(
                    out=out[:, GB * C_AP * b:GB * C_AP * (b + 1)],
                    in_=g[:].rearrange("p t c -> p (t c)"),
                )

    nc.compile()
    return nc


def _get_program():
    if "nc" not in _prog_cache:
        _prog_cache["nc"] = _build_program()
    return _prog_cache["nc"]


def kernel(z, ln_gamma, ln_beta, W, flat_atom_res_index, edge_index):
    z = np.asarray(z)
    ln_gamma = np.asarray(ln_gamma, dtype=np.float32)
    ln_beta = np.asarray(ln_beta, dtype=np.float32)
    W = np.asarray(W, dtype=np.float32)
    fari = np.asarray(flat_atom_res_index)
    ei = np.asarray(edge_index)

    n_batch, n_res, _, c_z = z.shape
    assert (n_batch, n_res, c_z) == (1, N_RES, C_Z)
    n_edges = ei.shape[1]

    zf = np.ascontiguousarray(z, dtype=np.float32).reshape(n_batch * n_res * n_res, c_z)

    # ------- host: constants -------
    wg = np.ascontiguousarray((ln_gamma[:, None] * W.T).astype(np.float32))  # [128,16]
    bw = (ln_beta @ W.T).astype(np.float32)                                  # [16]
    bw64 = np.ascontiguousarray(np.tile(bw, (128, 4)).astype(np.float32))    # [128,64]
    ident = np.eye(128, dtype=np.float32)

    # ------- host: bucket edges by r0-block -------
    r0 = fari[ei[0]].astype(np.int64)
    r1 = (fari[ei[1]].astype(np.int64)) % n_res
    core_of = (r0 >> 6).astype(np.int64)          # 64 rows per core
    order = np.argsort(core_of, kind="stable")
    counts = np.bincount(core_of, minlength=N_CORES)
    starts = np.zeros(N_CORES + 1, dtype=np.int64)
    np.cumsum(counts, out=starts[1:])

    r_local = ((r0 & 63) * n_res + r1).astype(np.int32)   # [0, 32768)

    in_maps = []
    edge_ids = []
    overflow = []
    for c in range(N_CORES):
        sel = order[starts[c]:starts[c + 1]]
        if len(sel) > N_PAD:
            overflow.append(sel[N_PAD:])
            sel = sel[:N_PAD]
        edge_ids.append(sel)
        ib = np.zeros(N_PAD, dtype=np.int32)
        ib[:len(sel)] = r_local[sel]
        # edge j = k*128 + p  ->  eidx[p, k]
        in_maps.append({
            "zs": np.ascontiguousarray(zf[c * ROWS_PER_CORE:(c + 1) * ROWS_PER_CORE]),
            "wg": wg,
            "bw64": bw64,
            "ident": ident,
            "eidx": np.ascontiguousarray(ib.reshape(K_TOT, 128).T),
        })

    nc = _get_program()
    res = bass_utils.run_bass_kernel_spmd(nc, in_maps, core_ids=list(range(N_CORES)))
    global _LAST_RES
    _LAST_RES = res

    out_full = np.empty((n_edges, C_AP), dtype=np.float32)
    for c in range(N_CORES):
        sel = edge_ids[c]
        # device out: [128, K_TOT*16]; edge j = k*128 + p at out[p, k*16:+16]
        dv = res.results[c]["out"].reshape(128, K_TOT, C_AP)
        dv = dv.transpose(1, 0, 2).reshape(N_PAD, C_AP)
        out_full[sel] = dv[:len(sel)]

    # host fallback for bucket overflow (normally empty)
    for sel in overflow:
        rows = zf[r0[sel] * n_res + r1[sel]].astype(np.float64)
        mu = rows.mean(axis=1, keepdims=True)
        var = rows.var(axis=1)
        xn = (rows - mu) / np.sqrt(var + LN_EPS)[:, None]
        out_full[sel] = (xn @ wg.astype(np.float64) + bw).astype(np.float32)

    return out_full


# revision 32
# speedup vs baseline: 1.4125x; 1.3753x over previous
"""Trainium2 Bass kernel for nn_BroadcastEdgeUpdate.

reference computes:
    res_edge_index = flat_atom_res_index[edge_index]           # [2, E]
    flatish_z      = z.reshape(R, n_res, c_z)                  # R = n_batch*n_res
    update         = einsum('rsc,ac->rsa', LN(flatish_z), W)   # [R, n_res, 16]
    out            = update[res_edge_index[0], res_edge_index[1] % n_res]

Sharding: core i owns table rows r0 in [64*i, 64*i+64) (z first-dim shard).
Edges are bucketed on the host by r0-block so each core gathers only from
its own locally-computed 2 MB table slice; the host undoes the permutation.

Device pipeline per core:
  phase A: z slice [32768, 128] --DMA--> bn_stats (DVE) -> rstd (ACT sqrt +
           DVE recip) -> fused (x-mu)*r (DVE tensor_scalar) -> PE transpose ->
           PE matmul with Wg = gamma*W^T -> PE transpose back -> +beta@W^T ->
           row-major [32768, 16] table in DRAM
  phase B: 1056 indirect-DMA instructions, each gathering 128 table rows
           (one 64 B row per partition, walrus consumes one offset per
           partition), batched into contiguous DMA-outs.
           (dma_gather/DMAGatherAnt would do 8k rows/inst but silently moves
           no data on this axon/fake_nrt runtime; ap_gather works but runs
           ~110 ns/idx on the Q7s — both rejected.)
"""

import numpy as np

import concourse.bass as bass
import concourse.bacc as bacc
import concourse.mybir as mybir
import concourse.tile as tile
from concourse import bass_utils
from concourse.bass import IndirectOffsetOnAxis

N_CORES = 8
N_RES = 512
C_Z = 128
C_AP = 16
ROWS_PER_CORE = (N_RES // N_CORES) * N_RES  # 32768 table rows
K_PAIR = 352                                # pair-gather insts (2 rows/descriptor)
K_SING = 336                                # single-gather insts
PAIR_CAP = K_PAIR * 128                     # 45056 pairs
SING_CAP = K_SING * 128                     # 43008 singles
GB = 16                                     # gather insts batched per output DMA
SG_ROWS = 4096                              # rows per super-group (32 tiles)
N_SG = ROWS_PER_CORE // SG_ROWS             # 8
LN_EPS = 1e-5
DEBUG_TABLE = False

_prog_cache = {}


def _build_program():
    f32 = mybir.dt.float32
    i32 = mybir.dt.int32
    nc = bacc.Bacc("TRN2", target_bir_lowering=False, debug=False,
                   num_devices=N_CORES)

    zs = nc.dram_tensor("zs", [ROWS_PER_CORE, C_Z], f32, kind="ExternalInput").ap()
    wg = nc.dram_tensor("wg", [C_Z, C_AP], f32, kind="ExternalInput").ap()
    bw64 = nc.dram_tensor("bw64", [128, 4 * C_AP], f32, kind="ExternalInput").ap()
    ident = nc.dram_tensor("ident", [128, 128], f32, kind="ExternalInput").ap()
    eidx = nc.dram_tensor("eidx", [128, K_PAIR + K_SING], i32,
                          kind="ExternalInput").ap()
    # pairs region then singles region; pair/single j = k*128 + p
    out = nc.dram_tensor("out", [128, (2 * K_PAIR + K_SING) * C_AP], f32,
                         kind="ExternalOutput").ap()
    table_dbg = (nc.dram_tensor("table_dbg", [ROWS_PER_CORE, C_AP], f32,
                                kind="ExternalOutput").ap()
                 if DEBUG_TABLE else None)

    with tile.TileContext(nc) as tc:
        with (
            tc.tile_pool(name="const", bufs=1) as cpool,
            tc.tile_pool(name="xin", bufs=2) as xpool,
            tc.tile_pool(name="xn", bufs=2) as xnpool,
            tc.tile_pool(name="stat", bufs=2) as spool,
            tc.tile_pool(name="xnt", bufs=3) as tpool,
            tc.tile_pool(name="u", bufs=3) as upool,
            tc.tile_pool(name="ostage", bufs=2) as opool,
            tc.tile_pool(name="psumT", bufs=2, space="PSUM") as ptpool,
            tc.tile_pool(name="psumA", bufs=2, space="PSUM") as papool,
            tc.tile_pool(name="psum2", bufs=2, space="PSUM") as p2pool,
            tc.tile_pool(name="gidx", bufs=1) as gipool,
            tc.tile_pool(name="gout", bufs=4) as gopool,
            tc.tile_pool(name="tbl", bufs=1, space="DRAM") as dpool,
        ):
            wg_t = cpool.tile([C_Z, C_AP], f32)
            nc.sync.dma_start(out=wg_t[:], in_=wg[:, :])
            bw_t = cpool.tile([128, 4 * C_AP], f32)
            nc.sync.dma_start(out=bw_t[:], in_=bw64[:, :])
            id_t = cpool.tile([128, 128], f32)
            nc.sync.dma_start(out=id_t[:], in_=ident[:, :])

            table = dpool.tile([ROWS_PER_CORE, C_AP], f32)

            # ---------------- phase A: build the update table ----------------
            for sg in range(N_SG):
                x = xpool.tile([128, 32, C_Z], f32, tag="x")
                rows = zs[sg * SG_ROWS:(sg + 1) * SG_ROWS, :]
                nc.sync.dma_start(out=x[:], in_=rows.rearrange("(t p) c -> p t c", p=128))

                stats = spool.tile([128, 32, 6], f32, tag="stats")
                for t in range(32):
                    nc.vector.bn_stats(out=stats[:, t, :], in_=x[:, t, :])

                # combine even/odd stats: n=128, ce=co=64
                # var = (M2e + M2o + 32*(me-mo)^2)/128 ; mean = (me+mo)/2
                t1 = spool.tile([128, 32, 1], f32, tag="t1")
                t2 = spool.tile([128, 32, 1], f32, tag="t2")
                t3 = spool.tile([128, 32, 1], f32, tag="t3")
                sd = spool.tile([128, 32, 1], f32, tag="sd")
                rr = spool.tile([128, 32, 1], f32, tag="rr")
                ms = spool.tile([128, 32, 1], f32, tag="ms")
                nc.vector.tensor_tensor(out=t1[:], in0=stats[:, :, 1:2],
                                        in1=stats[:, :, 4:5],
                                        op=mybir.AluOpType.subtract)
                nc.vector.tensor_tensor(out=t2[:], in0=stats[:, :, 2:3],
                                        in1=stats[:, :, 5:6],
                                        op=mybir.AluOpType.add)
                nc.vector.tensor_tensor(out=t3[:], in0=t1[:], in1=t1[:],
                                        op=mybir.AluOpType.mult)
                # t3 <- 32*t3 + 128*eps, then += t2  == 128*(var + eps)
                nc.vector.tensor_scalar(out=t3[:], in0=t3[:], scalar1=32.0,
                                        scalar2=float(C_Z * LN_EPS),
                                        op0=mybir.AluOpType.mult,
                                        op1=mybir.AluOpType.add)
                nc.vector.tensor_tensor(out=t3[:], in0=t3[:], in1=t2[:],
                                        op=mybir.AluOpType.add)
                nc.scalar.activation(out=sd[:], in_=t3[:],
                                     func=mybir.ActivationFunctionType.Sqrt,
                                     bias=0.0, scale=1.0 / C_Z)
                nc.vector.reciprocal(out=rr[:], in_=sd[:])
                nc.vector.tensor_tensor(out=ms[:], in0=stats[:, :, 1:2],
                                        in1=stats[:, :, 4:5],
                                        op=mybir.AluOpType.add)
                nc.vector.tensor_scalar(out=ms[:], in0=ms[:], scalar1=0.5,
                                        scalar2=None, op0=mybir.AluOpType.mult)

                xn = xnpool.tile([128, 32, C_Z], f32, tag="xn")
                for t in range(32):
                    nc.vector.tensor_scalar(out=xn[:, t, :], in0=x[:, t, :],
                                            scalar1=ms[:, t, :],
                                            scalar2=rr[:, t, :],
                                            op0=mybir.AluOpType.subtract,
                                            op1=mybir.AluOpType.mult)

                ostage = opool.tile([128, 32, C_AP], f32, tag="ostage")
                for gg in range(8):
                    psum_t = ptpool.tile([128, 512], f32, tag="pt")
                    for t4 in range(4):
                        nc.tensor.transpose(out=psum_t[:, 128 * t4:128 * (t4 + 1)],
                                            in_=xn[:, 4 * gg + t4, :],
                                            identity=id_t[:])
                    xnt = tpool.tile([128, 512], f32, tag="xnt")
                    if gg % 2 == 0:
                        nc.vector.tensor_copy(out=xnt[:], in_=psum_t[:])
                    else:
                        nc.scalar.copy(out=xnt[:], in_=psum_t[:])
                    psum_a = papool.tile([C_AP, 512], f32, tag="pa")
                    nc.tensor.matmul(out=psum_a[:], lhsT=wg_t[:], rhs=xnt[:],
                                     start=True, stop=True)
                    u = upool.tile([C_AP, 512], f32, tag="u")
                    nc.scalar.copy(out=u[:], in_=psum_a[:])
                    psum_2 = p2pool.tile([128, 4 * C_AP], f32, tag="p2")
                    for t4 in range(4):
                        nc.tensor.transpose(out=psum_2[:, C_AP * t4:C_AP * (t4 + 1)],
                                            in_=u[:, 128 * t4:128 * (t4 + 1)],
                                            identity=id_t[:C_AP, :C_AP])
                    nc.vector.tensor_tensor(out=ostage[:, 4 * gg:4 * gg + 4, :],
                                            in0=psum_2[:].rearrange("p (t c) -> p t c", t=4),
                                            in1=bw_t[:].rearrange("p (t c) -> p t c", t=4),
                                            op=mybir.AluOpType.add)
                rows_out = table[sg * SG_ROWS:(sg + 1) * SG_ROWS, :]
                nc.sync.dma_start(
                    out=rows_out.rearrange("(t p) c -> p t c", p=128),
                    in_=ostage[:])
                if table_dbg is not None:
                    dbg_rows = table_dbg[sg * SG_ROWS:(sg + 1) * SG_ROWS, :]
                    nc.sync.dma_start(
                        out=dbg_rows.rearrange("(t p) c -> p t c", p=128),
                        in_=ostage[:])

            # ---------------- phase B: gather the edges ----------------
            # indirect DMA: one descriptor per partition per instruction.
            # pair insts fetch a contiguous run of 2 rows (idx even);
            # single insts fetch 1 row.
            idx_all = gipool.tile([128, K_PAIR + K_SING], i32)
            nc.sync.dma_start(out=idx_all[:], in_=eidx[:, :])
            for b in range(K_PAIR // GB):
                g = gopool.tile([128, GB, 2 * C_AP], f32, tag="gout2")
                for t in range(GB):
                    k = b * GB + t
                    nc.gpsimd.indirect_dma_start(
                        out=g[:, t, :],
                        out_offset=None,
                        in_=table[:, :],
                        in_offset=IndirectOffsetOnAxis(
                            ap=idx_all[:, k:k + 1], axis=0),
                    )
                nc.sync.dma_start(
                    out=out[:, 2 * GB * C_AP * b:2 * GB * C_AP * (b + 1)],
                    in_=g[:].rearrange("p t c -> p (t c)"),
                )
            OFF = 2 * K_PAIR * C_AP
            for b in range(K_SING // GB):
                g = gopool.tile([128, GB, C_AP], f32, tag="gout1")
                for t in range(GB):
                    k = K_PAIR + b * GB + t
                    nc.gpsimd.indirect_dma_start(
                        out=g[:, t, :],
                        out_offset=None,
                        in_=table[:, :],
                        in_offset=IndirectOffsetOnAxis(
                            ap=idx_all[:, k:k + 1], axis=0),
                    )
                nc.sync.dma_start(
                    out=out[:, OFF + GB * C_AP * b:OFF + GB * C_AP * (b + 1)],
                    in_=g[:].rearrange("p t c -> p (t c)"),
                )

    nc.compile()
    return nc


def _get_program():
    if "nc" not in _prog_cache:
        _prog_cache["nc"] = _build_program()
    return _prog_cache["nc"]


def kernel(z, ln_gamma, ln_beta, W, flat_atom_res_index, edge_index):
    z = np.asarray(z)
    ln_gamma = np.asarray(ln_gamma, dtype=np.float32)
    ln_beta = np.asarray(ln_beta, dtype=np.float32)
    W = np.asarray(W, dtype=np.float32)
    fari = np.asarray(flat_atom_res_index)
    ei = np.asarray(edge_index)

    n_batch, n_res, _, c_z = z.shape
    assert (n_batch, n_res, c_z) == (1, N_RES, C_Z)
    n_edges = ei.shape[1]

    zf = np.ascontiguousarray(z, dtype=np.float32).reshape(n_batch * n_res * n_res, c_z)

    # ------- host: constants -------
    wg = np.ascontiguousarray((ln_gamma[:, None] * W.T).astype(np.float32))  # [128,16]
    bw = (ln_beta @ W.T).astype(np.float32)                                  # [16]
    bw64 = np.ascontiguousarray(np.tile(bw, (128, 4)).astype(np.float32))    # [128,64]
    ident = np.eye(128, dtype=np.float32)

    # ------- host: bucket edges by r0-block -------
    r0 = fari[ei[0]].astype(np.int64)
    r1 = (fari[ei[1]].astype(np.int64)) % n_res
    core_of = (r0 >> 6).astype(np.int64)          # 64 rows per core
    order = np.argsort(core_of, kind="stable")
    counts = np.bincount(core_of, minlength=N_CORES)
    starts = np.zeros(N_CORES + 1, dtype=np.int64)
    np.cumsum(counts, out=starts[1:])

    r_local = ((r0 & 63) * n_res + r1).astype(np.int32)   # [0, 32768)

    in_maps = []
    pair_ids = []   # per core: (pairA edge ids, pairB edge ids)
    sing_ids = []   # per core: single edge ids
    overflow = []
    for c in range(N_CORES):
        sel = order[starts[c]:starts[c + 1]]
        rows = r_local[sel]
        ordr = np.argsort(rows, kind="stable")
        es = sel[ordr]
        rs = rows[ordr].astype(np.int64)
        cnt = np.bincount(rs, minlength=ROWS_PER_CORE)
        off = np.zeros(ROWS_PER_CORE + 1, dtype=np.int64)
        np.cumsum(cnt, out=off[1:])
        # even/odd row pairing: block k pairs rows (2k, 2k+1)
        npk = np.minimum(cnt[0::2], cnt[1::2])
        csum = np.cumsum(npk)
        npk = np.clip(PAIR_CAP - (csum - npk), 0, npk)  # cap total pairs
        tp = int(npk.sum())
        K = np.repeat(np.arange(ROWS_PER_CORE // 2), npk)
        st = np.concatenate([[0], np.cumsum(npk)[:-1]])
        I = np.arange(tp) - np.repeat(st, npk)
        pA = es[off[2 * K] + I]
        pB = es[off[2 * K + 1] + I]
        pair_ids.append((pA, pB))
        # singles: per-row leftovers after pairing
        used = np.repeat(npk, 2)
        left = cnt - used
        ts = int(left.sum())
        R = np.repeat(np.arange(ROWS_PER_CORE), left)
        stl = np.concatenate([[0], np.cumsum(left)[:-1]])
        J = np.arange(ts) - np.repeat(stl, left)
        sE = es[off[R] + used[R] + J]
        if len(sE) > SING_CAP:
            overflow.append(sE[SING_CAP:])
            sE = sE[:SING_CAP]
        sing_ids.append(sE)
        ibp = np.zeros(PAIR_CAP, dtype=np.int32)
        ibp[:tp] = (2 * K).astype(np.int32)
        ibs = np.zeros(SING_CAP, dtype=np.int32)
        ibs[:len(sE)] = r_local[sE]
        eidx_arr = np.concatenate(
            [ibp.reshape(K_PAIR, 128).T, ibs.reshape(K_SING, 128).T], axis=1)
        in_maps.append({
            "zs": np.ascontiguousarray(zf[c * ROWS_PER_CORE:(c + 1) * ROWS_PER_CORE]),
            "wg": wg,
            "bw64": bw64,
            "ident": ident,
            "eidx": np.ascontiguousarray(eidx_arr),
        })

    nc = _get_program()
    res = bass_utils.run_bass_kernel_spmd(nc, in_maps, core_ids=list(range(N_CORES)))
    global _LAST_RES
    _LAST_RES = res

    out_full = np.empty((n_edges, C_AP), dtype=np.float32)
    for c in range(N_CORES):
        dv = res.results[c]["out"]
        P = dv[:, :2 * K_PAIR * C_AP].reshape(128, K_PAIR, 2, C_AP)
        P = P.transpose(1, 0, 2, 3).reshape(PAIR_CAP, 2, C_AP)
        pA, pB = pair_ids[c]
        out_full[pA] = P[:len(pA), 0]
        out_full[pB] = P[:len(pB), 1]
        S = dv[:, 2 * K_PAIR * C_AP:].reshape(128, K_SING, C_AP)
        S = S.transpose(1, 0, 2).reshape(SING_CAP, C_AP)
        sE = sing_ids[c]
        out_full[sE] = S[:len(sE)]

    # host fallback for bucket overflow (normally empty)
    for sel in overflow:
        rows = zf[r0[sel] * n_res + r1[sel]].astype(np.float64)
        mu = rows.mean(axis=1, keepdims=True)
        var = rows.var(axis=1)
        xn = (rows - mu) / np.sqrt(var + LN_EPS)[:, None]
        out_full[sel] = (xn @ wg.astype(np.float64) + bw).astype(np.float32)

    return out_full


# revision 38
# speedup vs baseline: 1.6104x; 1.1401x over previous
"""Trainium2 Bass kernel for nn_BroadcastEdgeUpdate.

reference computes:
    res_edge_index = flat_atom_res_index[edge_index]           # [2, E]
    flatish_z      = z.reshape(R, n_res, c_z)                  # R = n_batch*n_res
    update         = einsum('rsc,ac->rsa', LN(flatish_z), W)   # [R, n_res, 16]
    out            = update[res_edge_index[0], res_edge_index[1] % n_res]

Sharding: core i owns table rows r0 in [64*i, 64*i+64) (z first-dim shard).
Edges are bucketed on the host by r0-block so each core gathers only from
its own locally-computed 2 MB table slice; the host undoes the permutation.

Device pipeline per core:
  phase A: z slice [32768, 128] --DMA--> bn_stats (DVE) -> rstd (ACT sqrt +
           DVE recip) -> fused (x-mu)*r (DVE tensor_scalar) -> PE transpose ->
           PE matmul with Wg = gamma*W^T -> PE transpose back -> +beta@W^T ->
           row-major [32768, 16] table in DRAM
  phase B: indirect-DMA gather, one descriptor per partition per
           instruction (walrus consumes one offset per partition; each
           descriptor copies a contiguous run). The host pairs edges whose
           table rows are (2k, 2k+1) so one descriptor serves two edges
           (a 128 B run); leftovers go through single-row instructions.
           352 pair insts + 336 single insts = 688 vs 1024 unpaired.
           (dma_gather/DMAGatherAnt would do 8k rows/inst but silently moves
           no data on this axon/fake_nrt runtime; ap_gather works but runs
           ~110 ns/idx on the Q7s — both rejected.)
"""

import numpy as np

import concourse.bass as bass
import concourse.bacc as bacc
import concourse.mybir as mybir
import concourse.tile as tile
from concourse import bass_utils
from concourse.bass import IndirectOffsetOnAxis

N_CORES = 8
N_RES = 512
C_Z = 128
C_AP = 16
ROWS_PER_CORE = (N_RES // N_CORES) * N_RES  # 32768 table rows
K_QUAD = 128                                # quad-gather insts (4 rows/descriptor)
K_PAIR = 128                                # pair-gather insts (2 rows/descriptor)
K_SING = 320                                # single-gather insts
QUAD_CAP = K_QUAD * 128                     # 16384 quads
PAIR_CAP = K_PAIR * 128                     # 16384 pairs
SING_CAP = K_SING * 128                     # 40960 singles
GB = 16                                     # gather insts batched per output DMA
SG_ROWS = 4096                              # rows per super-group (32 tiles)
N_SG = ROWS_PER_CORE // SG_ROWS             # 8
LN_EPS = 1e-5
DEBUG_TABLE = False

_prog_cache = {}


def _build_program():
    f32 = mybir.dt.float32
    i32 = mybir.dt.int32
    nc = bacc.Bacc("TRN2", target_bir_lowering=False, debug=False,
                   num_devices=N_CORES)

    zs = nc.dram_tensor("zs", [ROWS_PER_CORE, C_Z], f32, kind="ExternalInput").ap()
    wg = nc.dram_tensor("wg", [C_Z, C_AP], f32, kind="ExternalInput").ap()
    bw64 = nc.dram_tensor("bw64", [128, 4 * C_AP], f32, kind="ExternalInput").ap()
    ident = nc.dram_tensor("ident", [128, 128], f32, kind="ExternalInput").ap()
    eidx = nc.dram_tensor("eidx", [128, K_QUAD + K_PAIR + K_SING], i32,
                          kind="ExternalInput").ap()
    # quads, then pairs, then singles; slot j = k*128 + p in each region
    out = nc.dram_tensor(
        "out", [128, (4 * K_QUAD + 2 * K_PAIR + K_SING) * C_AP], f32,
        kind="ExternalOutput").ap()
    table_dbg = (nc.dram_tensor("table_dbg", [ROWS_PER_CORE, C_AP], f32,
                                kind="ExternalOutput").ap()
                 if DEBUG_TABLE else None)

    with tile.TileContext(nc) as tc:
        with (
            tc.tile_pool(name="const", bufs=1) as cpool,
            tc.tile_pool(name="xin", bufs=2) as xpool,
            tc.tile_pool(name="xn", bufs=2) as xnpool,
            tc.tile_pool(name="stat", bufs=2) as spool,
            tc.tile_pool(name="xnt", bufs=3) as tpool,
            tc.tile_pool(name="u", bufs=3) as upool,
            tc.tile_pool(name="ostage", bufs=2) as opool,
            tc.tile_pool(name="psumT", bufs=2, space="PSUM") as ptpool,
            tc.tile_pool(name="psumA", bufs=2, space="PSUM") as papool,
            tc.tile_pool(name="psum2", bufs=2, space="PSUM") as p2pool,
            tc.tile_pool(name="gidx", bufs=1) as gipool,
            tc.tile_pool(name="gout", bufs=4) as gopool,
            tc.tile_pool(name="tbl", bufs=1, space="DRAM") as dpool,
        ):
            wg_t = cpool.tile([C_Z, C_AP], f32)
            nc.sync.dma_start(out=wg_t[:], in_=wg[:, :])
            bw_t = cpool.tile([128, 4 * C_AP], f32)
            nc.sync.dma_start(out=bw_t[:], in_=bw64[:, :])
            id_t = cpool.tile([128, 128], f32)
            nc.sync.dma_start(out=id_t[:], in_=ident[:, :])

            table = dpool.tile([ROWS_PER_CORE, C_AP], f32)

            # ---------------- phase A: build the update table ----------------
            for sg in range(N_SG):
                x = xpool.tile([128, 32, C_Z], f32, tag="x")
                rows = zs[sg * SG_ROWS:(sg + 1) * SG_ROWS, :]
                nc.sync.dma_start(out=x[:], in_=rows.rearrange("(t p) c -> p t c", p=128))

                stats = spool.tile([128, 32, 6], f32, tag="stats")
                for t in range(32):
                    nc.vector.bn_stats(out=stats[:, t, :], in_=x[:, t, :])

                # combine even/odd stats: n=128, ce=co=64
                # var = (M2e + M2o + 32*(me-mo)^2)/128 ; mean = (me+mo)/2
                t1 = spool.tile([128, 32, 1], f32, tag="t1")
                t2 = spool.tile([128, 32, 1], f32, tag="t2")
                t3 = spool.tile([128, 32, 1], f32, tag="t3")
                sd = spool.tile([128, 32, 1], f32, tag="sd")
                rr = spool.tile([128, 32, 1], f32, tag="rr")
                ms = spool.tile([128, 32, 1], f32, tag="ms")
                nc.vector.tensor_tensor(out=t1[:], in0=stats[:, :, 1:2],
                                        in1=stats[:, :, 4:5],
                                        op=mybir.AluOpType.subtract)
                nc.vector.tensor_tensor(out=t2[:], in0=stats[:, :, 2:3],
                                        in1=stats[:, :, 5:6],
                                        op=mybir.AluOpType.add)
                nc.vector.tensor_tensor(out=t3[:], in0=t1[:], in1=t1[:],
                                        op=mybir.AluOpType.mult)
                # t3 <- 32*t3 + 128*eps, then += t2  == 128*(var + eps)
                nc.vector.tensor_scalar(out=t3[:], in0=t3[:], scalar1=32.0,
                                        scalar2=float(C_Z * LN_EPS),
                                        op0=mybir.AluOpType.mult,
                                        op1=mybir.AluOpType.add)
                nc.vector.tensor_tensor(out=t3[:], in0=t3[:], in1=t2[:],
                                        op=mybir.AluOpType.add)
                nc.scalar.activation(out=sd[:], in_=t3[:],
                                     func=mybir.ActivationFunctionType.Sqrt,
                                     bias=0.0, scale=1.0 / C_Z)
                nc.vector.reciprocal(out=rr[:], in_=sd[:])
                nc.vector.tensor_tensor(out=ms[:], in0=stats[:, :, 1:2],
                                        in1=stats[:, :, 4:5],
                                        op=mybir.AluOpType.add)
                nc.vector.tensor_scalar(out=ms[:], in0=ms[:], scalar1=0.5,
                                        scalar2=None, op0=mybir.AluOpType.mult)

                xn = xnpool.tile([128, 32, C_Z], f32, tag="xn")
                for t in range(32):
                    nc.vector.tensor_scalar(out=xn[:, t, :], in0=x[:, t, :],
                                            scalar1=ms[:, t, :],
                                            scalar2=rr[:, t, :],
                                            op0=mybir.AluOpType.subtract,
                                            op1=mybir.AluOpType.mult)

                ostage = opool.tile([128, 32, C_AP], f32, tag="ostage")
                for gg in range(8):
                    psum_t = ptpool.tile([128, 512], f32, tag="pt")
                    for t4 in range(4):
                        nc.tensor.transpose(out=psum_t[:, 128 * t4:128 * (t4 + 1)],
                                            in_=xn[:, 4 * gg + t4, :],
                                            identity=id_t[:])
                    xnt = tpool.tile([128, 512], f32, tag="xnt")
                    if gg % 2 == 0:
                        nc.vector.tensor_copy(out=xnt[:], in_=psum_t[:])
                    else:
                        nc.scalar.copy(out=xnt[:], in_=psum_t[:])
                    psum_a = papool.tile([C_AP, 512], f32, tag="pa")
                    nc.tensor.matmul(out=psum_a[:], lhsT=wg_t[:], rhs=xnt[:],
                                     start=True, stop=True)
                    u = upool.tile([C_AP, 512], f32, tag="u")
                    nc.scalar.copy(out=u[:], in_=psum_a[:])
                    psum_2 = p2pool.tile([128, 4 * C_AP], f32, tag="p2")
                    for t4 in range(4):
                        nc.tensor.transpose(out=psum_2[:, C_AP * t4:C_AP * (t4 + 1)],
                                            in_=u[:, 128 * t4:128 * (t4 + 1)],
                                            identity=id_t[:C_AP, :C_AP])
                    nc.vector.tensor_tensor(out=ostage[:, 4 * gg:4 * gg + 4, :],
                                            in0=psum_2[:].rearrange("p (t c) -> p t c", t=4),
                                            in1=bw_t[:].rearrange("p (t c) -> p t c", t=4),
                                            op=mybir.AluOpType.add)
                rows_out = table[sg * SG_ROWS:(sg + 1) * SG_ROWS, :]
                nc.sync.dma_start(
                    out=rows_out.rearrange("(t p) c -> p t c", p=128),
                    in_=ostage[:])
                if table_dbg is not None:
                    dbg_rows = table_dbg[sg * SG_ROWS:(sg + 1) * SG_ROWS, :]
                    nc.sync.dma_start(
                        out=dbg_rows.rearrange("(t p) c -> p t c", p=128),
                        in_=ostage[:])

            # ---------------- phase B: gather the edges ----------------
            # indirect DMA: one descriptor per partition per instruction.
            # pair insts fetch a contiguous run of 2 rows (idx even);
            # single insts fetch 1 row.
            idx_all = gipool.tile([128, K_QUAD + K_PAIR + K_SING], i32)
            nc.sync.dma_start(out=idx_all[:], in_=eidx[:, :])

            def gather_region(k0, n_inst, rows_per, out_off, tag):
                w = rows_per * C_AP
                for b in range(n_inst // GB):
                    g = gopool.tile([128, GB, w], f32, tag=tag)
                    for t in range(GB):
                        k = k0 + b * GB + t
                        nc.gpsimd.indirect_dma_start(
                            out=g[:, t, :],
                            out_offset=None,
                            in_=table[:, :],
                            in_offset=IndirectOffsetOnAxis(
                                ap=idx_all[:, k:k + 1], axis=0),
                        )
                    nc.sync.dma_start(
                        out=out[:, out_off + GB * w * b:out_off + GB * w * (b + 1)],
                        in_=g[:].rearrange("p t c -> p (t c)"),
                    )

            gather_region(0, K_QUAD, 4, 0, "gout4")
            gather_region(K_QUAD, K_PAIR, 2, 4 * K_QUAD * C_AP, "gout2")
            gather_region(K_QUAD + K_PAIR, K_SING, 1,
                          (4 * K_QUAD + 2 * K_PAIR) * C_AP, "gout1")

    nc.compile()
    return nc


def _get_program():
    if "nc" not in _prog_cache:
        _prog_cache["nc"] = _build_program()
    return _prog_cache["nc"]


def kernel(z, ln_gamma, ln_beta, W, flat_atom_res_index, edge_index):
    z = np.asarray(z)
    ln_gamma = np.asarray(ln_gamma, dtype=np.float32)
    ln_beta = np.asarray(ln_beta, dtype=np.float32)
    W = np.asarray(W, dtype=np.float32)
    fari = np.asarray(flat_atom_res_index)
    ei = np.asarray(edge_index)

    n_batch, n_res, _, c_z = z.shape
    assert (n_batch, n_res, c_z) == (1, N_RES, C_Z)
    n_edges = ei.shape[1]

    zf = np.ascontiguousarray(z, dtype=np.float32).reshape(n_batch * n_res * n_res, c_z)

    # ------- host: constants -------
    wg = np.ascontiguousarray((ln_gamma[:, None] * W.T).astype(np.float32))  # [128,16]
    bw = (ln_beta @ W.T).astype(np.float32)                                  # [16]
    bw64 = np.ascontiguousarray(np.tile(bw, (128, 4)).astype(np.float32))    # [128,64]
    ident = np.eye(128, dtype=np.float32)

    # ------- host: bucket edges by r0-block -------
    r0 = fari[ei[0]].astype(np.int64)
    r1 = (fari[ei[1]].astype(np.int64)) % n_res
    core_of = (r0 >> 6).astype(np.int64)          # 64 rows per core
    order = np.argsort(core_of, kind="stable")
    counts = np.bincount(core_of, minlength=N_CORES)
    starts = np.zeros(N_CORES + 1, dtype=np.int64)
    np.cumsum(counts, out=starts[1:])

    r_local = ((r0 & 63) * n_res + r1).astype(np.int32)   # [0, 32768)

    def _take(avail, cap):
        """cap per-block counts so the running total stays <= cap"""
        cs = np.cumsum(avail)
        return np.clip(cap - (cs - avail), 0, avail)

    def _expand(nblk, per_blk):
        tot = int(per_blk.sum())
        K = np.repeat(np.arange(nblk), per_blk)
        st = np.concatenate([[0], np.cumsum(per_blk)[:-1]])
        I = np.arange(tot) - np.repeat(st, per_blk)
        return K, I

    in_maps = []
    quad_ids = []   # per core: list of 4 edge-id arrays
    pair_ids = []   # per core: (pairA, pairB)
    sing_ids = []   # per core: single edge ids
    overflow = []
    for c in range(N_CORES):
        sel = order[starts[c]:starts[c + 1]]
        rows = r_local[sel]
        ordr = np.argsort(rows, kind="stable")
        es = sel[ordr]
        rs = rows[ordr].astype(np.int64)
        cnt = np.bincount(rs, minlength=ROWS_PER_CORE)
        off = np.zeros(ROWS_PER_CORE + 1, dtype=np.int64)
        np.cumsum(cnt, out=off[1:])
        # tier 1: quads over row blocks (4k..4k+3)
        nq = np.minimum.reduce([cnt[0::4], cnt[1::4], cnt[2::4], cnt[3::4]])
        nq = _take(nq, QUAD_CAP)
        tq = int(nq.sum())
        K4, I4 = _expand(ROWS_PER_CORE // 4, nq)
        qE = [es[off[4 * K4 + u] + I4] for u in range(4)]
        quad_ids.append(qE)
        offp = off[:ROWS_PER_CORE] + np.repeat(nq, 4)
        left = cnt - np.repeat(nq, 4)
        # tier 2: pairs over row blocks (2k, 2k+1)
        npk = _take(np.minimum(left[0::2], left[1::2]), PAIR_CAP)
        tp = int(npk.sum())
        K2, I2 = _expand(ROWS_PER_CORE // 2, npk)
        pA = es[offp[2 * K2] + I2]
        pB = es[offp[2 * K2 + 1] + I2]
        pair_ids.append((pA, pB))
        offs = offp + np.repeat(npk, 2)
        lefts = left - np.repeat(npk, 2)
        # tier 3: singles
        ts = int(lefts.sum())
        R, J = _expand(ROWS_PER_CORE, lefts)
        sE = es[offs[R] + J]
        if len(sE) > SING_CAP:
            overflow.append(sE[SING_CAP:])
            sE = sE[:SING_CAP]
        sing_ids.append(sE)
        ibq = np.zeros(QUAD_CAP, dtype=np.int32)
        ibq[:tq] = (4 * K4).astype(np.int32)
        ibp = np.zeros(PAIR_CAP, dtype=np.int32)
        ibp[:tp] = (2 * K2).astype(np.int32)
        ibs = np.zeros(SING_CAP, dtype=np.int32)
        ibs[:len(sE)] = r_local[sE]
        eidx_arr = np.concatenate(
            [ibq.reshape(K_QUAD, 128).T, ibp.reshape(K_PAIR, 128).T,
             ibs.reshape(K_SING, 128).T], axis=1)
        in_maps.append({
            "zs": np.ascontiguousarray(zf[c * ROWS_PER_CORE:(c + 1) * ROWS_PER_CORE]),
            "wg": wg,
            "bw64": bw64,
            "ident": ident,
            "eidx": np.ascontiguousarray(eidx_arr),
        })

    nc = _get_program()
    res = bass_utils.run_bass_kernel_spmd(nc, in_maps, core_ids=list(range(N_CORES)))
    global _LAST_RES
    _LAST_RES = res

    out_full = np.empty((n_edges, C_AP), dtype=np.float32)
    QW = 4 * K_QUAD * C_AP
    PW = 2 * K_PAIR * C_AP
    for c in range(N_CORES):
        dv = res.results[c]["out"]
        Q = dv[:, :QW].reshape(128, K_QUAD, 4, C_AP)
        Q = Q.transpose(1, 0, 2, 3).reshape(QUAD_CAP, 4, C_AP)
        for u in range(4):
            qe = quad_ids[c][u]
            out_full[qe] = Q[:len(qe), u]
        P = dv[:, QW:QW + PW].reshape(128, K_PAIR, 2, C_AP)
        P = P.transpose(1, 0, 2, 3).reshape(PAIR_CAP, 2, C_AP)
        pA, pB = pair_ids[c]
        out_full[pA] = P[:len(pA), 0]
        out_full[pB] = P[:len(pB), 1]
        S = dv[:, QW + PW:].reshape(128, K_SING, C_AP)
        S = S.transpose(1, 0, 2).reshape(SING_CAP, C_AP)
        sE = sing_ids[c]
        out_full[sE] = S[:len(sE)]

    # host fallback for bucket overflow (normally empty)
    for sel in overflow:
        rows = zf[r0[sel] * n_res + r1[sel]].astype(np.float64)
        mu = rows.mean(axis=1, keepdims=True)
        var = rows.var(axis=1)
        xn = (rows - mu) / np.sqrt(var + LN_EPS)[:, None]
        out_full[sel] = (xn @ wg.astype(np.float64) + bw).astype(np.float32)

    return out_full
